# revision 4
# baseline (speedup 1.0000x reference)
"""Trainium2 Bass kernel for CustomPointScatter (nn_CustomPointScatter).

Reference computation:
    pillar_feat = point_features.mean(axis=1)            # [40000, 64]
    out = zeros([4, 64, 512, 512]); out[b, :, y, x] = pillar_feat

Strategy (data parallel over pillars, 8 cores):
  - Host casts point_features to fp16 (tolerance is 2e-2; fp16 cast error on
    a mean-of-32 is ~1e-4 relative) and hands core r the contiguous pillar
    slice [r*5000, (r+1)*5000), zero-padded to 5120 rows.
  - Each core streams its [5120, 2048] fp16 slab through SBUF in tiles of
    512 pillars (4 pillar rows per partition -> 16 KB contiguous per-partition
    DMA descriptors on the SP hardware-DGE ring) and reduces the 32-point axis
    with five in-place halving DVE adds (fp16 gets the 2x packed mode).  The
    final add writes a compact [128, 4*64] fp16 feature tile that goes back to
    DRAM on the ACT hardware-DGE ring (contiguous 64 KB per tile), giving a
    per-core [5120, 64] mean tensor.
  - The host unshard step upcasts to f32, applies the 1/32 mean scale (exact
    in binary), and places rows into the dense [4, 64, 512, 512] output at
    (b, :, y, x) -- pure data movement, like the baseline's bank-sum+transpose
    assembly, but cheaper.

Per-core HW floor: 20.97 MB fp16 read at ~358 GB/s = ~59 us; DVE add tree
~48 us fully overlapped; writes 0.66 MB on the second DGE ring.
"""

import numpy as np

import concourse.bacc as bacc
import concourse.mybir as mybir
import concourse.tile as tile
from concourse.bass_utils import run_bass_kernel_spmd

B, H, W = 4, 512, 512
N_PILLARS, N_POINTS, C = 40000, 32, 64
N_CORES = 8
P = 128                  # SBUF partitions
D = N_POINTS * C         # 2048 fp16 elems per pillar row
NPC = N_PILLARS // N_CORES   # 5000 pillars per core
IPB = 4                  # pillar rows per partition per tile
TPT = P * IPB            # 512 pillars per tile
NMAX = 5120              # NPC padded up to a multiple of TPT
NT = NMAX // TPT         # 10 tiles
BUFS = 7
GP_W = 128               # stages with w <= GP_W run on GpSimd (DVE offload)


def build_nc(nmax=NMAX, ipb=IPB, bufs=BUFS, gp_w=GP_W):
    tpt = P * ipb
    nt = nmax // tpt
    nc = bacc.Bacc("TRN2", target_bir_lowering=False)
    pf = nc.dram_tensor("pf", [nmax, D], mybir.dt.float16, kind="ExternalInput")
    po = nc.dram_tensor("po", [nmax, C], mybir.dt.float16, kind="ExternalOutput")
    with tile.TileContext(nc) as tc:
        with (
            tc.tile_pool(name="io", bufs=bufs) as io_pool,
            tc.tile_pool(name="fo", bufs=4) as fo_pool,
        ):
            for t in range(nt):
                rows = slice(t * tpt, (t + 1) * tpt)
                sb = io_pool.tile([P, ipb * D], mybir.dt.float16, tag="sb")
                v = sb[:].rearrange("p (i w) -> p i w", w=D)
                # pillar j = t*tpt + p*ipb + i -> partition p, block i:
                # 16 KB contiguous per partition on both sides.
                nc.sync.dma_start(
                    out=v,
                    in_=pf[rows, :].rearrange("(p i) w -> p i w", p=P),
                )
                # halving-add reduce over the 32-point axis; big stages on
                # DVE (2x packed fp16), small ones offloaded to GpSimd.
                w = D // 2
                while w > C:
                    eng = nc.vector if w > gp_w else nc.gpsimd
                    eng.tensor_add(
                        out=v[:, :, :w], in0=v[:, :, :w], in1=v[:, :, w : 2 * w]
                    )
                    w //= 2
                feat = fo_pool.tile([P, ipb * C], mybir.dt.float16, tag="feat")
                fv = feat[:].rearrange("p (i w) -> p i w", w=C)
                eng = nc.vector if C > gp_w else nc.gpsimd
                eng.tensor_add(out=fv, in0=v[:, :, :C], in1=v[:, :, C : 2 * C])
                # write the whole tile's means: contiguous tpt*C*2 bytes.
                nc.scalar.dma_start(
                    out=po[rows, :].rearrange("(p i) w -> p i w", p=P),
                    in_=fv,
                )
    nc.finalize()
    return nc


def shard_inputs(point_features):
    pf = np.asarray(point_features, dtype=np.float32).reshape(N_PILLARS, D)
    pf16 = pf.astype(np.float16)
    in_maps = []
    for r in range(N_CORES):
        pf_r = np.zeros((NMAX, D), np.float16)
        pf_r[:NPC] = pf16[r * NPC : (r + 1) * NPC]
        in_maps.append({"pf": pf_r})
    return in_maps


def assemble(results, voxel_coords):
    vc = np.asarray(voxel_coords)
    b = vc[:, 0].astype(np.int64)
    y = vc[:, 2].astype(np.int64)
    x = vc[:, 3].astype(np.int64)
    out = np.zeros((B, C, H, W), np.float32)
    inv_np = np.float32(1.0 / N_POINTS)
    for r in range(N_CORES):
        sl = slice(r * NPC, (r + 1) * NPC)
        feats = results[r]["po"][:NPC].astype(np.float32) * inv_np
        out[b[sl], :, y[sl], x[sl]] = feats
    return out


def run(point_features, voxel_coords, trace=False, ipb=IPB, bufs=BUFS,
        gp_w=GP_W, **spmd_kwargs):
    in_maps = shard_inputs(point_features)
    nc = build_nc(ipb=ipb, bufs=bufs, gp_w=gp_w)
    br = run_bass_kernel_spmd(
        nc, in_maps, list(range(N_CORES)), trace=trace, **spmd_kwargs
    )
    return assemble(br.results, voxel_coords), br


def kernel(point_features, voxel_coords):
    out, _ = run(point_features, voxel_coords)
    return out


# revision 5
# speedup vs baseline: 1.0448x; 1.0448x over previous
"""Trainium2 Bass kernel for CustomPointScatter (nn_CustomPointScatter).

Reference computation:
    pillar_feat = point_features.mean(axis=1)            # [40000, 64]
    out = zeros([4, 64, 512, 512]); out[b, :, y, x] = pillar_feat

Strategy (data parallel over pillars, 8 cores):
  - Host casts point_features to fp16 (tolerance is 2e-2; fp16 cast error on
    a mean-of-32 is ~1e-4 relative) and hands core r the contiguous pillar
    slice [r*5000, (r+1)*5000), zero-padded to 5120 rows.
  - Each core streams its [5120, 2048] fp16 slab through SBUF in tiles of
    512 pillars (4 pillar rows per partition -> 16 KB contiguous per-partition
    DMA descriptors on the SP hardware-DGE ring) and reduces the 32-point axis
    with five in-place halving DVE adds (fp16 gets the 2x packed mode).  The
    final add writes a compact [128, 4*64] fp16 feature tile that goes back to
    DRAM on the ACT hardware-DGE ring (contiguous 64 KB per tile), giving a
    per-core [5120, 64] mean tensor.
  - The host unshard step upcasts to f32, applies the 1/32 mean scale (exact
    in binary), and places rows into the dense [4, 64, 512, 512] output at
    (b, :, y, x) -- pure data movement, like the baseline's bank-sum+transpose
    assembly, but cheaper.

Per-core HW floor: 20.97 MB fp16 read at ~358 GB/s = ~59 us; DVE add tree
~48 us fully overlapped; writes 0.66 MB on the second DGE ring.
"""

import numpy as np

import concourse.bacc as bacc
import concourse.mybir as mybir
import concourse.tile as tile
from concourse.bass_utils import run_bass_kernel_spmd

B, H, W = 4, 512, 512
N_PILLARS, N_POINTS, C = 40000, 32, 64
N_CORES = 8
P = 128                  # SBUF partitions
D = N_POINTS * C         # 2048 fp16 elems per pillar row
NPC = N_PILLARS // N_CORES   # 5000 pillars per core
IPB = 4                  # pillar rows per partition per tile
TPT = P * IPB            # 512 pillars per tile
NMAX = 5120              # NPC padded up to a multiple of TPT
NT = NMAX // TPT         # 10 tiles
BUFS = 7
GP_W = 0                 # stages with w <= GP_W run on GpSimd (0: all on DVE)


def build_nc(nmax=NMAX, ipb=IPB, bufs=BUFS, gp_w=GP_W):
    tpt = P * ipb
    nt = nmax // tpt
    nc = bacc.Bacc("TRN2", target_bir_lowering=False)
    pf = nc.dram_tensor("pf", [nmax, D], mybir.dt.float16, kind="ExternalInput")
    po = nc.dram_tensor("po", [nmax, C], mybir.dt.float16, kind="ExternalOutput")
    with tile.TileContext(nc) as tc:
        with (
            tc.tile_pool(name="io", bufs=bufs) as io_pool,
            tc.tile_pool(name="fo", bufs=4) as fo_pool,
        ):
            for t in range(nt):
                rows = slice(t * tpt, (t + 1) * tpt)
                sb = io_pool.tile([P, ipb * D], mybir.dt.float16, tag="sb")
                v = sb[:].rearrange("p (i w) -> p i w", w=D)
                # pillar j = t*tpt + p*ipb + i -> partition p, block i:
                # 16 KB contiguous per partition on both sides.
                nc.sync.dma_start(
                    out=v,
                    in_=pf[rows, :].rearrange("(p i) w -> p i w", p=P),
                )
                # halving-add reduce over the 32-point axis; big stages on
                # DVE (2x packed fp16), small ones offloaded to GpSimd.
                w = D // 2
                while w > C:
                    eng = nc.vector if w > gp_w else nc.gpsimd
                    eng.tensor_add(
                        out=v[:, :, :w], in0=v[:, :, :w], in1=v[:, :, w : 2 * w]
                    )
                    w //= 2
                feat = fo_pool.tile([P, ipb * C], mybir.dt.float16, tag="feat")
                fv = feat[:].rearrange("p (i w) -> p i w", w=C)
                eng = nc.vector if C > gp_w else nc.gpsimd
                eng.tensor_add(out=fv, in0=v[:, :, :C], in1=v[:, :, C : 2 * C])
                # write the whole tile's means: contiguous tpt*C*2 bytes.
                nc.scalar.dma_start(
                    out=po[rows, :].rearrange("(p i) w -> p i w", p=P),
                    in_=fv,
                )
    nc.finalize()
    return nc


def shard_inputs(point_features):
    pf = np.asarray(point_features, dtype=np.float32).reshape(N_PILLARS, D)
    pf16 = pf.astype(np.float16)
    in_maps = []
    for r in range(N_CORES):
        pf_r = np.zeros((NMAX, D), np.float16)
        pf_r[:NPC] = pf16[r * NPC : (r + 1) * NPC]
        in_maps.append({"pf": pf_r})
    return in_maps


def assemble(results, voxel_coords):
    vc = np.asarray(voxel_coords)
    b = vc[:, 0].astype(np.int64)
    y = vc[:, 2].astype(np.int64)
    x = vc[:, 3].astype(np.int64)
    out = np.zeros((B, C, H, W), np.float32)
    inv_np = np.float32(1.0 / N_POINTS)
    for r in range(N_CORES):
        sl = slice(r * NPC, (r + 1) * NPC)
        feats = results[r]["po"][:NPC].astype(np.float32) * inv_np
        out[b[sl], :, y[sl], x[sl]] = feats
    return out


def run(point_features, voxel_coords, trace=False, ipb=IPB, bufs=BUFS,
        gp_w=GP_W, **spmd_kwargs):
    in_maps = shard_inputs(point_features)
    nc = build_nc(ipb=ipb, bufs=bufs, gp_w=gp_w)
    br = run_bass_kernel_spmd(
        nc, in_maps, list(range(N_CORES)), trace=trace, **spmd_kwargs
    )
    return assemble(br.results, voxel_coords), br


def kernel(point_features, voxel_coords):
    out, _ = run(point_features, voxel_coords)
    return out


# revision 6
# speedup vs baseline: 1.0765x; 1.0303x over previous
"""Trainium2 Bass kernel for CustomPointScatter (nn_CustomPointScatter).

Reference computation:
    pillar_feat = point_features.mean(axis=1)            # [40000, 64]
    out = zeros([4, 64, 512, 512]); out[b, :, y, x] = pillar_feat

Strategy (data parallel over pillars, 8 cores):
  - Host casts point_features to fp16 (tolerance is 2e-2; fp16 error on a
    mean-of-32 is ~1e-3 relative) and hands core r the contiguous pillar
    slice [r*5000, (r+1)*5000) -- a zero-copy view, no padding.
  - Each core streams its [5000, 2048] fp16 slab through SBUF in tiles
    (4 pillar rows per partition -> 16 KB contiguous per-partition DMA
    descriptors on the SP hardware-DGE ring) and reduces the 32-point axis
    with five in-place halving DVE adds (fp16 2x packed mode).  The final
    add writes a compact fp16 feature tile that returns to DRAM on the ACT
    hardware-DGE ring, giving a per-core [5000, 64] mean tensor.
  - The 5000-row slab is covered by nine 512-row tiles plus four 128-row
    tail tiles (the last overlapping by 120 rows) so the pipeline drains
    through small tiles: the post-last-load DVE chain is ~1.6 us, not ~5.
  - The host unshard step upcasts to f32, applies the 1/32 mean scale
    (exact in binary), and places rows into the dense [4, 64, 512, 512]
    output at (b, :, y, x).

Per-core hardware profile: the 16 SDMA engines each carry 1/16 of the
20.5 MB load at ~26 GB/s line rate (~48 us), overlapped with ~49 us of DVE
adds; prologue ~8 us is framework-fixed.
"""

import numpy as np

import concourse.bacc as bacc
import concourse.mybir as mybir
import concourse.tile as tile
from concourse.bass_utils import run_bass_kernel_spmd

B, H, W = 4, 512, 512
N_PILLARS, N_POINTS, C = 40000, 32, 64
N_CORES = 8
P = 128                  # SBUF partitions
D = N_POINTS * C         # 2048 fp16 elems per pillar row
NPC = N_PILLARS // N_CORES   # 5000 pillars per core
IPB = 4                  # pillar rows per partition per (full) tile
BUFS = 7


def tile_plan(npc=NPC, ipb=IPB):
    """(start, ipb) per tile: full tiles then ipb=1 tail tiles; the final
    tail tile is shifted back so every row < npc is covered exactly."""
    full = P * ipb
    plan = []
    pos = 0
    while pos + full <= npc:
        plan.append((pos, ipb))
        pos += full
    while pos + P <= npc:
        plan.append((pos, 1))
        pos += P
    if pos < npc:
        plan.append((npc - P, 1))
    return plan


def build_nc(npc=NPC, ipb=IPB, bufs=BUFS):
    plan = tile_plan(npc, ipb)
    nc = bacc.Bacc("TRN2", target_bir_lowering=False)
    pf = nc.dram_tensor("pf", [npc, D], mybir.dt.float16, kind="ExternalInput")
    po = nc.dram_tensor("po", [npc, C], mybir.dt.float16, kind="ExternalOutput")
    with tile.TileContext(nc) as tc:
        with (
            tc.tile_pool(name="io", bufs=bufs) as io_pool,
            tc.tile_pool(name="fo", bufs=4) as fo_pool,
        ):
            for start, tipb in plan:
                rows = slice(start, start + P * tipb)
                sb = io_pool.tile([P, tipb * D], mybir.dt.float16, tag="sb")
                v = sb[:].rearrange("p (i w) -> p i w", w=D)
                # pillar j = start + p*tipb + i -> partition p, block i:
                # tipb*4 KB contiguous per partition on both sides.
                nc.sync.dma_start(
                    out=v,
                    in_=pf[rows, :].rearrange("(p i) w -> p i w", p=P),
                )
                w = D // 2
                while w > C:
                    nc.vector.tensor_add(
                        out=v[:, :, :w], in0=v[:, :, :w], in1=v[:, :, w : 2 * w]
                    )
                    w //= 2
                feat = fo_pool.tile([P, tipb * C], mybir.dt.float16, tag="feat")
                fv = feat[:].rearrange("p (i w) -> p i w", w=C)
                nc.vector.tensor_add(
                    out=fv, in0=v[:, :, :C], in1=v[:, :, C : 2 * C]
                )
                # write the tile's means: contiguous P*tipb*C*2 bytes.
                nc.scalar.dma_start(
                    out=po[rows, :].rearrange("(p i) w -> p i w", p=P),
                    in_=fv,
                )
    nc.finalize()
    return nc


def shard_inputs(point_features):
    pf = np.asarray(point_features, dtype=np.float32).reshape(N_PILLARS, D)
    pf16 = pf.astype(np.float16)
    return [{"pf": pf16[r * NPC : (r + 1) * NPC]} for r in range(N_CORES)]


def assemble(results, voxel_coords):
    vc = np.asarray(voxel_coords)
    b = vc[:, 0].astype(np.int64)
    y = vc[:, 2].astype(np.int64)
    x = vc[:, 3].astype(np.int64)
    out = np.zeros((B, C, H, W), np.float32)
    inv_np = np.float32(1.0 / N_POINTS)
    for r in range(N_CORES):
        sl = slice(r * NPC, (r + 1) * NPC)
        feats = results[r]["po"].astype(np.float32) * inv_np
        out[b[sl], :, y[sl], x[sl]] = feats
    return out


def run(point_features, voxel_coords, trace=False, ipb=IPB, bufs=BUFS,
        **spmd_kwargs):
    in_maps = shard_inputs(point_features)
    nc = build_nc(ipb=ipb, bufs=bufs)
    br = run_bass_kernel_spmd(
        nc, in_maps, list(range(N_CORES)), trace=trace, **spmd_kwargs
    )
    return assemble(br.results, voxel_coords), br


def kernel(point_features, voxel_coords):
    out, _ = run(point_features, voxel_coords)
    return out
